# revision 1
# baseline (speedup 1.0000x reference)
"""Trainium2 Bass kernel for the prototype-bank scatter-mean EMA update
(nn_Bank): data-parallel over N across 8 NeuronCores.

Per core:
  1. Zero a DRAM accumulator acc[32*1024, 128] f32 (32 replica banks; row
     rep*1024 + c = class c in replica rep; row *+1000 = dump row for padding
     tokens; cols 0..63 = feature sums, col 64 = count).
  2. Stream feature chunks (S=2048 tokens) HBM->SBUF with a constant 1.0
     column appended, and dma_scatter_add each chunk into acc.
     The HW scatter-add loses updates when two in-flight descriptors target
     the same address, so the host assigns each token a replica index
     r = occurrence-rank of its class within the chunk (verified < 16), and
     consecutive chunks (at most 2 in flight) use disjoint replica halves:
     idx = ((chunk%2)*16 + r)*1024 + label. All addresses touched by the
     <=2 in-flight scatters are therefore unique.
  3. Reduce the 32 replica banks on-chip (SBUF adds) -> acc2[1024, 128].
  4. AllReduce acc2 across the 8 cores.
  5. Compute means + EMA blend on-chip, write out[1000, 64].

The host only shards inputs, reformats labels into the scatter's int16
"wrapped" index layout (including the replica rank), and picks core 0's
output.
"""

import numpy as np

import concourse.bacc as bacc
import concourse.bass as bass
import concourse.mybir as mybir
from concourse import bass_utils

C = 1000
D = 64
BANK = 1024      # rows per replica bank (1000 classes + dump + pad)
R_HALF = 16      # replica banks per in-flight window
NWIN = 3         # concurrent scatter windows (one per async SWDGE queue)
R_TOT = NWIN * R_HALF
ACC_ROWS = R_TOT * BANK
ACC_W = 128      # acc row stride in f32 elements (512B, multiple of 256B)
ELEM = D + 1     # 64 feature cols + 1 count col
LAM = 0.9
WARMUP_STEP = 1000
N_CORES = 8
S_MAIN = 2048


def plan_chunks(t_core: int, s_main: int):
    assert s_main % 128 == 0
    n_main = t_core // s_main
    rem = t_core - n_main * s_main
    if rem == 0:
        return n_main, 0, t_core
    s_tail = ((rem + 127) // 128) * 128
    return n_main, s_tail, n_main * s_main + s_tail


def host_labels_to_idx(labels: np.ndarray, s_main: int, s_tail: int) -> np.ndarray:
    """int16 [128, t_pad//16]; chunk i occupies columns [i*S/16, (i+1)*S/16).

    Scatter token j of a chunk (G = S//128) is sample (j%128)*G + j//128 (the
    feature DMA loads the chunk contiguously, partition p holding samples
    p*G..p*G+G-1); its idx sits at [j%16, j//16], replicated across the 8
    groups of 16 partitions.

    idx value = ((chunk%2)*R_HALF + r)*BANK + label, where r is the token's
    occurrence rank of its label within the chunk (must be < R_HALF).
    Padding tokens get the dump class C with r = position%R_HALF (collisions
    there only lose dump-row data).
    """
    n_main = len(labels) // s_main
    t_pad = n_main * s_main + s_tail
    lab = np.full(t_pad, C, dtype=np.int64)
    lab[: len(labels)] = labels
    sizes = [s_main] * n_main + ([s_tail] if s_tail else [])

    # occurrence rank of each token within its (chunk, label) group
    chunk_id = np.minimum(np.arange(t_pad) // s_main, len(sizes) - 1)
    key = chunk_id * (C + 24) + lab
    order = np.argsort(key, kind="stable")
    sk = key[order]
    starts = np.flatnonzero(np.r_[True, sk[1:] != sk[:-1]])
    group_len = np.diff(np.r_[starts, len(sk)])
    rank_sorted = np.arange(len(sk)) - np.repeat(starts, group_len)
    r = np.empty(t_pad, np.int64)
    r[order] = rank_sorted
    pad_mask = lab == C
    r[pad_mask] = np.arange(t_pad)[pad_mask] % R_HALF
    if r.max() >= R_HALF:
        raise ValueError(f"replica overflow: max rank {r.max()} >= {R_HALF}")
    # window base comes from the per-chunk out_ap offset on device
    idx = r * BANK + lab
    assert idx.max() < 2 ** 15
    idx = idx.astype(np.int16)

    cols = []
    off = 0
    for S in sizes:
        G = S // 128
        chunk = idx[off : off + S]
        off += S
        slot = chunk.reshape(128, G).T.ravel()
        tile16 = slot.reshape(S // 16, 16).T
        cols.append(np.tile(tile16, (8, 1)))
    return np.concatenate(cols, axis=1)


def build_nc(n_cores: int, t_core: int, s_main: int, step: int, stage: int = 3):
    n_main, s_tail, t_pad = plan_chunks(t_core, s_main)
    G = s_main // 128
    G_t = s_tail // 128
    sizes = [(s_main, G)] * n_main + ([(s_tail, G_t)] if s_tail else [])
    n_chunks = len(sizes)
    NB = 6   # feature tile buffers
    RB = 4   # replica-reduce buffers

    f32 = mybir.dt.float32
    i16 = mybir.dt.int16

    nc = bacc.Bacc("TRN2", target_bir_lowering=False, debug=False,
                   num_devices=n_cores, num_swdge_queues=4)

    feat = nc.dram_tensor("feature", [n_main * s_main, D], f32, kind="ExternalInput")
    if s_tail:
        feat_tail = nc.dram_tensor("feature_tail", [s_tail, D], f32, kind="ExternalInput")
    labels = nc.dram_tensor("labels_idx", [128, t_pad // 16], i16, kind="ExternalInput")
    proto = nc.dram_tensor("prototype", [C, D], f32, kind="ExternalInput")
    out = nc.dram_tensor("out", [C, D], f32, kind="ExternalOutput")

    CC_W = 72  # collective row width: 65 used cols + pad (vs ACC_W=128)
    acc2 = nc.dram_tensor("acc2", [BANK, CC_W], f32)
    acc_red = nc.dram_tensor("acc_red", [BANK, CC_W], f32)

    ftiles = [nc.alloc_sbuf_tensor(f"ftile{b}", [128, G * ELEM], f32) for b in range(NB)]
    ltiles = [nc.alloc_sbuf_tensor(f"ltile{b}", [128, G * D], f32) for b in range(NB)]
    lab_sb = nc.alloc_sbuf_tensor("lab_sb", [128, t_pad // 16], i16)
    # SBUF accumulators, parity-split (tpr=128, dhi=1, dlo=ELEM):
    # id = r*BANK + c -> partition c%128, parity bit7(c), group r*4 + (c>>8);
    # one 64-group window per in-flight scatter.
    GRP = R_HALF * 4
    sacc_own = nc.alloc_sbuf_tensor("sacc_own", [128, NWIN * GRP * ELEM], f32)
    sacc_peer = nc.alloc_sbuf_tensor("sacc_peer", [128, NWIN * GRP * ELEM], f32)
    red_own = nc.alloc_sbuf_tensor("red_own", [128, 4 * ACC_W], f32)
    red_peer = nc.alloc_sbuf_tensor("red_peer", [128, 4 * ACC_W], f32)
    asum = nc.alloc_sbuf_tensor("asum", [128, 8 * ELEM], f32)
    ptile = nc.alloc_sbuf_tensor("ptile", [128, 8 * D], f32)
    otile = nc.alloc_sbuf_tensor("otile", [128, 8 * D], f32)
    cntm = nc.alloc_sbuf_tensor("cntm", [128, 8], f32)
    rcp = nc.alloc_sbuf_tensor("rcp", [128, 8], f32)
    pres = nc.alloc_sbuf_tensor("pres", [128, 8], f32)
    znorm = nc.alloc_sbuf_tensor("znorm", [128, 8], f32)
    unew = nc.alloc_sbuf_tensor("unew", [128, 8], f32)
    means = nc.alloc_sbuf_tensor("means", [128, 8 * D], f32)
    tmp = nc.alloc_sbuf_tensor("tmp", [128, 8 * D], f32)

    init_sem = nc.alloc_semaphore("init_sem")
    zacc_sem = nc.alloc_semaphore("zacc_sem")
    lab_sem = nc.alloc_semaphore("lab_sem")
    load_sems = [nc.alloc_semaphore(f"load_sem{b}") for b in range(NB)]
    rs_sem = nc.alloc_semaphore("rs_sem")
    ssems = [nc.alloc_semaphore(f"ssem{p}") for p in range(NWIN)]
    rload_sems = [nc.alloc_semaphore(f"rload_sem{b}") for b in range(RB)]
    radd_sem = nc.alloc_semaphore("radd_sem")
    racc_sem = nc.alloc_semaphore("racc_sem")
    cc_sem = nc.alloc_semaphore("cc_sem")
    ld2_sem = nc.alloc_semaphore("ld2_sem")
    comp_sem = nc.alloc_semaphore("comp_sem")
    vch = nc.alloc_semaphore("vch")

    def ftile_ap3(b, g_cnt):
        t = ftiles[b]
        return bass.AP(t, 0, [[t.ap().ap[0][0], 128], [ELEM, g_cnt], [1, ELEM]])

    def ftile_feat_ap(b, g_cnt):
        t = ftiles[b]
        return bass.AP(t, 0, [[t.ap().ap[0][0], 128], [ELEM, g_cnt], [1, D]])

    def ftile_ones_ap(b, g_cnt):
        t = ftiles[b]
        return bass.AP(t, D, [[t.ap().ap[0][0], 128], [ELEM, g_cnt], [1, 1]])

    def ltile_ap(b, g_cnt):
        t = ltiles[b]
        return bass.AP(t, 0, [[t.ap().ap[0][0], 128], [D, g_cnt], [1, D]])

    def feat_chunk_ap(i):
        S, Gc = sizes[i]
        if i < n_main:
            return bass.AP(feat, i * s_main * D, [[Gc * D, 128], [D, Gc], [1, D]])
        return bass.AP(feat_tail, 0, [[Gc * D, 128], [D, Gc], [1, D]])

    def chunk_col_off(i):
        return sum(sz // 16 for sz, _ in sizes[:i])

    def acc_bank_flat_ap(rep):
        # replica bank `rep` as a flat [128, 1024]-shaped DMA view
        return bass.AP(acc, rep * BANK * ACC_W, [[BANK * ACC_W // 128, 128], [1, BANK * ACC_W // 128]])

    with nc.Block() as block:

        @block.vector
        def _(vector):
            vector.memset(sacc_own.ap(), 0.0).then_inc(init_sem, 1)
            vector.memset(sacc_peer.ap(), 0.0).then_inc(init_sem, 1)
            for b in range(NB):
                vector.memset(ftile_ones_ap(b, G), 1.0).then_inc(init_sem, 1)
            vector.memset(ptile.ap(), 0.0).then_inc(init_sem, 1)
            vector.memset(red_own.ap(), 0.0).then_inc(init_sem, 1)
            vector.memset(red_peer.ap(), 0.0).then_inc(init_sem, 1)

        @block.sync
        def _(sync):
            sync.dma_start(lab_sb.ap(), labels.ap()).then_inc(lab_sem, 16)
            for i in range(min(NB, n_chunks)):
                sync.dma_start(ltile_ap(i, sizes[i][1]), feat_chunk_ap(i)).then_inc(load_sems[i], 16)
            for i, (S, Gc) in enumerate(sizes):
                if i < NB:
                    continue
                b = i % NB
                # ltile b was consumed by restripe of chunk i-NB
                sync.wait_ge(rs_sem, i - NB + 1)
                sync.dma_start(ltile_ap(b, Gc), feat_chunk_ap(i)).then_inc(load_sems[b], 16)
            sync.wait_ge(init_sem, NB + 3)  # ptile memset done
            sync.dma_start(
                bass.AP(ptile, 0, [[ptile.ap().ap[0][0], 128], [D, 7], [1, D]]),
                bass.AP(proto, 0, [[D, 128], [128 * D, 7], [1, D]]),
            ).then_inc(ld2_sem, 16)
            sync.dma_start(
                bass.AP(ptile, 7 * D, [[ptile.ap().ap[0][0], C - 896], [1, D]]),
                bass.AP(proto, 896 * D, [[D, C - 896], [1, D]]),
            ).then_inc(ld2_sem, 16)

        @block.vector
        def _(vector):
            # restripe: contiguous ltile -> 65-strided ftile (fat DMA loads,
            # per-token-contiguous scatter source)
            for i, (S, Gc) in enumerate(sizes):
                b = i % NB
                vector.wait_ge(load_sems[b], 16 * (i // NB + 1))
                if i >= NB:
                    j = i - NB  # scatter that last read this ftile
                    vector.wait_ge(ssems[j % NWIN], 16 * (j // NWIN + 1))
                vector.tensor_copy(ftile_feat_ap(b, Gc), ltile_ap(b, Gc)).then_inc(rs_sem, 1)

        @block.gpsimd
        def _(gpsimd):
            gpsimd.wait_ge(lab_sem, 16)
            gpsimd.wait_ge(init_sem, 2 + NB)
            for i, (S, Gc) in enumerate(sizes):
                b = i % NB
                gpsimd.wait_ge(rs_sem, i + 1)
                if i >= NWIN:
                    # at most NWIN scatters in flight (disjoint windows)
                    gpsimd.wait_ge(ssems[i % NWIN], 16 * (i // NWIN))
                co = chunk_col_off(i)
                idx_ap = bass.AP(lab_sb, co, [[lab_sb.ap().ap[0][0], 128], [1, S // 16]])
                w = i % NWIN
                own_ap = bass.AP(sacc_own, w * GRP * ELEM,
                                 [[sacc_own.ap().ap[0][0], 128], [1, GRP * ELEM]])
                peer_ap = bass.AP(sacc_peer, w * GRP * ELEM,
                                  [[sacc_peer.ap().ap[0][0], 128], [1, GRP * ELEM]])
                gpsimd.dma_scatter_add(
                    out_ap=own_ap,
                    in_ap=ftile_ap3(b, Gc),
                    idxs_ap=idx_ap,
                    num_idxs=S,
                    num_idxs_reg=S,
                    elem_size=ELEM,
                    sbuf_tokens_per_rank=128,
                    parity_reg=0,
                    out_ap_other=peer_ap,
                    single_packet=False,
                    queue_num=1 + (i % NWIN),  # queue 0 desc-gen is synchronous on Pool; 1..3 async
                ).then_inc(ssems[i % NWIN], 16)

        # ---- replica reduce: sacc windows/ranks -> red (SBUF) -> acc2 ----
        @block.vector
        def _(vector):
            vector.wait_ge(init_sem, NB + 5)  # red tiles memset (same-engine WAW)
            for p in range(NWIN):
                k = n_chunks - 1 - ((n_chunks - 1 - p) % NWIN)  # last chunk of window p
                if k >= 0:
                    vector.wait_ge(ssems[k % NWIN], 16 * (k // NWIN + 1))
            for t, o in ((sacc_own, red_own), (sacc_peer, red_peer)):
                # element (p, w, r, k, d) at offset w*GRP*ELEM + (r*4+k)*ELEM + d;
                # reduce over (w, r) -> out [128, 4, ELEM]
                vector.tensor_reduce(
                    bass.AP(o, 0, [[o.ap().ap[0][0], 128], [ACC_W, 4], [1, ELEM]]),
                    bass.AP(t, 0, [[t.ap().ap[0][0], 128], [ELEM, 4], [1, ELEM],
                                   [GRP * ELEM, NWIN], [4 * ELEM, R_HALF]]),
                    axis=mybir.AxisListType.XY,
                    op=mybir.AluOpType.add,
                ).then_inc(radd_sem, 1)

        @block.sync
        def _(sync):
            sync.wait_ge(radd_sem, 2)
            # class c = k*256 + parity*128 + p  ->  acc2 row c, cols 0..ELEM
            sync.dma_start(
                bass.AP(acc2, 0, [[CC_W, 128], [256 * CC_W, 4], [1, CC_W]]),
                bass.AP(red_own, 0, [[red_own.ap().ap[0][0], 128], [ACC_W, 4], [1, CC_W]]),
            ).then_inc(racc_sem, 16)
            sync.dma_start(
                bass.AP(acc2, 128 * CC_W, [[CC_W, 128], [256 * CC_W, 4], [1, CC_W]]),
                bass.AP(red_peer, 0, [[red_peer.ap().ap[0][0], 128], [ACC_W, 4], [1, CC_W]]),
            ).then_inc(racc_sem, 16)

        @block.gpsimd
        def _(gpsimd):
            gpsimd.wait_ge(racc_sem, 32)
            if stage >= 2:
                gpsimd.collective_compute(
                    "AllReduce",
                    mybir.AluOpType.add,
                    replica_groups=[list(range(n_cores))],
                    ins=[acc2.ap().opt()],
                    outs=[acc_red.ap().opt()],
                ).then_inc(cc_sem, 1)
            else:
                gpsimd.nop().then_inc(cc_sem, 1)

        # ---- blend phase ----
        acc_src = acc_red if stage >= 2 else acc2

        @block.sync
        def _(sync):
            sync.wait_ge(cc_sem, 1)
            sync.dma_start(
                bass.AP(asum, 0, [[asum.ap().ap[0][0], 128], [ELEM, 8], [1, ELEM]]),
                bass.AP(acc_src, 0, [[CC_W, 128], [128 * CC_W, 8], [1, ELEM]]),
            ).then_inc(ld2_sem, 16)

        @block.vector
        def _(vector):
            vector.wait_ge(ld2_sem, 48)
            if stage < 3:
                for g in range(8):
                    vector.tensor_copy(
                        bass.AP(otile, g * D, [[otile.ap().ap[0][0], 128], [1, D]]),
                        bass.AP(asum, g * ELEM, [[asum.ap().ap[0][0], 128], [1, D]]),
                    ).then_inc(comp_sem, 1)
                return
            vc = [0]

            def chain(ins):
                ins.then_inc(vch, 1)
                vc[0] += 1
                vector.wait_ge(vch, vc[0])

            ap_s = asum.ap()
            cnt_ap = bass.AP(asum, D, [[ap_s.ap[0][0], 128], [ELEM, 8], [1, 1]])
            chain(vector.tensor_copy(cntm.ap(), cnt_ap))
            chain(vector.tensor_scalar_max(rcp.ap(), cntm.ap(), 1.0))
            chain(vector.reciprocal(rcp.ap(), rcp.ap()))
            chain(vector.tensor_scalar(pres.ap(), cntm.ap(), 0.0, None, mybir.AluOpType.is_gt))
            chain(vector.tensor_reduce(
                znorm.ap(),
                bass.AP(ptile, 0, [[ptile.ap().ap[0][0], 128], [D, 8], [1, D]]),
                axis=mybir.AxisListType.X,
                op=mybir.AluOpType.max,
                apply_absolute_value=True,
            ))
            if step <= WARMUP_STEP:
                chain(vector.memset(unew.ap(), 1.0))
            else:
                chain(vector.tensor_scalar(unew.ap(), znorm.ap(), 0.0, None, mybir.AluOpType.is_equal))
            for g in range(8):
                def col(t, w=D):
                    return bass.AP(t, g * w, [[t.ap().ap[0][0], 128], [1, w]])
                def colsum(t):
                    return bass.AP(t, g, [[t.ap().ap[0][0], 128], [1, 1]])
                sums_g = bass.AP(asum, g * ELEM, [[ap_s.ap[0][0], 128], [1, D]])
                chain(vector.tensor_scalar_mul(col(means), sums_g, colsum(rcp)))
                chain(vector.tensor_scalar_mul(col(otile), col(ptile), LAM))
                chain(vector.tensor_scalar_mul(col(tmp), col(means), 1.0 - LAM))
                chain(vector.tensor_add(col(otile), col(otile), col(tmp)))
                chain(vector.tensor_sub(col(tmp), col(means), col(otile)))
                chain(vector.tensor_scalar_mul(col(tmp), col(tmp), colsum(unew)))
                chain(vector.tensor_add(col(otile), col(otile), col(tmp)))
                chain(vector.tensor_sub(col(tmp), col(otile), col(ptile)))
                chain(vector.tensor_scalar_mul(col(tmp), col(tmp), colsum(pres)))
                vector.tensor_add(col(otile), col(ptile), col(tmp)).then_inc(comp_sem, 1)

        @block.sync
        def _(sync):
            sync.wait_ge(comp_sem, 8)
            sync.dma_start(
                bass.AP(out, 0, [[D, 128], [128 * D, 7], [1, D]]),
                bass.AP(otile, 0, [[otile.ap().ap[0][0], 128], [D, 7], [1, D]]),
            ).then_inc(ld2_sem, 16)
            sync.dma_start(
                bass.AP(out, 896 * D, [[D, C - 896], [1, D]]),
                bass.AP(otile, 7 * D, [[otile.ap().ap[0][0], C - 896], [1, D]]),
            ).then_inc(ld2_sem, 16)
            sync.wait_ge(ld2_sem, 80)

    nc.compile()
    return nc


def shard_inputs(feature, label, prototype, n_cores, t_core, s_main):
    n_main, s_tail, t_pad = plan_chunks(t_core, s_main)
    in_maps = []
    proto32 = np.ascontiguousarray(prototype, dtype=np.float32)
    for k in range(n_cores):
        lo = k * t_core
        hi = min((k + 1) * t_core, feature.shape[0])
        m = {
            "feature": np.ascontiguousarray(feature[lo : lo + n_main * s_main], dtype=np.float32),
            "labels_idx": host_labels_to_idx(np.asarray(label[lo:hi]), s_main, s_tail),
            "prototype": proto32,
        }
        if s_tail:
            ft = np.zeros((s_tail, D), dtype=np.float32)
            nt = hi - (lo + n_main * s_main)
            ft[:nt] = feature[lo + n_main * s_main : hi]
            m["feature_tail"] = ft
        in_maps.append(m)
    return in_maps


_NC_CACHE = {}


def run(inputs: dict, trace: bool = False, stage: int = 3):
    feature = np.asarray(inputs["feature"])
    label = np.asarray(inputs["label"])
    prototype = np.asarray(inputs["prototype"])
    step = int(np.asarray(inputs["step"]))

    n = feature.shape[0]
    assert n % N_CORES == 0, n
    t_core = n // N_CORES

    s_main = S_MAIN
    while True:
        try:
            in_maps = shard_inputs(feature, label, prototype, N_CORES, t_core, s_main)
            break
        except ValueError:
            # heavy label skew: smaller chunks bound the per-chunk duplicates
            s_main //= 2
            if s_main < 128:
                raise
    key = (t_core, s_main, step > WARMUP_STEP, stage)
    if key not in _NC_CACHE:
        _NC_CACHE[key] = build_nc(N_CORES, t_core, s_main, step, stage=stage)
    nc = _NC_CACHE[key]
    res = bass_utils.run_bass_kernel_spmd(
        nc, in_maps, core_ids=list(range(N_CORES)), trace=trace,
    )
    out = np.asarray(res.results[0]["out"], dtype=np.float32)
    return out, res


def kernel(**inputs) -> np.ndarray:
    out, _ = run(inputs, trace=False)
    return out



# revision 2
# speedup vs baseline: 5.3476x; 5.3476x over previous
"""Trainium2 Bass kernel for the prototype-bank scatter-mean EMA update
(nn_Bank): class-sharded sorted-segment reduction across 8 NeuronCores.

Host (index work only, all FP arithmetic stays on device):
  * argsort labels; assign each class to one core (greedy token balance,
    <=128 classes/core); concatenate each core's class segments, zero-
    padding every class to a multiple of GTOK tokens so each GTOK-token
    "block" is single-class.
  * per-core metadata: block -> local-class id and block valid-token
    count; prototype rows for the core's classes; an iota row table.

Device, per core (T tiles; tile = 128 blocks = 128*GTOK tokens):
  1. Stream feature tiles [128, GTOK*D] HBM->SBUF (two DMA queues).
  2. Vector: block sums via tensor_reduce -> rhs [128, D]; one-hot
     lhsT [128 blocks, 128 local classes] via iota==cls compare.
  3. PE: psum_s[cls, D] += oh^T @ block_sums ; psum_c[cls, 1] += oh^T @
     block_counts (PSUM accumulation over all T tiles).
  4. Blend: means = sums/max(cnt,1); out = proto + s*(means-proto),
     s = present * (0.1 + 0.9*use_new)  [step>warmup branch].
  5. DMA out [128, D]; host scatters per-core rows back to [1000, 64].

No collective: every class is fully owned by one core.
"""

import numpy as np

import concourse.bacc as bacc
import concourse.bass as bass
import concourse.mybir as mybir
from concourse import bass_utils

C = 1000
D = 64
P = 128
GTOK = 32            # tokens per block (class padding granularity)
LAM = 0.9
WARMUP_STEP = 1000
N_CORES = 8
NB = 6               # feature tile buffers (3 per DMA-issuing engine)
FEAT_MODE = "f32"    # "f32" | "bf16"


def _feat_np_dtype():
    if FEAT_MODE == "bf16":
        import ml_dtypes
        return ml_dtypes.bfloat16
    return np.float32


def _feat_bir_dtype():
    return mybir.dt.bfloat16 if FEAT_MODE == "bf16" else mybir.dt.float32


def build_nc(T: int, step_gt_warmup: bool):
    f32 = mybir.dt.float32
    fdt = _feat_bir_dtype()
    FW = GTOK * D  # free elems per feature tile partition

    nc = bacc.Bacc("TRN2", target_bir_lowering=False, debug=False,
                   num_devices=N_CORES)

    feat = nc.dram_tensor("feature", [T * P, FW], fdt, kind="ExternalInput")
    meta = nc.dram_tensor("blk_meta", [P, 2 * T], f32, kind="ExternalInput")
    proto = nc.dram_tensor("prototype", [P, D], f32, kind="ExternalInput")
    iota = nc.dram_tensor("iota", [P, P], f32, kind="ExternalInput")
    out = nc.dram_tensor("out", [P, D], f32, kind="ExternalOutput")

    ftiles = [nc.alloc_sbuf_tensor(f"ftile{b}", [P, FW], fdt) for b in range(NB)]
    iota_sb = nc.alloc_sbuf_tensor("iota_sb", [P, P], f32)
    meta_sb = nc.alloc_sbuf_tensor("meta_sb", [P, 2 * T], f32)
    proto_sb = nc.alloc_sbuf_tensor("proto_sb", [P, D], f32)
    rhs = [nc.alloc_sbuf_tensor(f"rhs{i}", [P, D], f32) for i in range(2)]
    oh = [nc.alloc_sbuf_tensor(f"oh{i}", [P, P], f32) for i in range(2)]
    cnt = nc.alloc_sbuf_tensor("cnt", [P, 1], f32)
    rcp = nc.alloc_sbuf_tensor("rcp", [P, 1], f32)
    pres = nc.alloc_sbuf_tensor("pres", [P, 1], f32)
    znorm = nc.alloc_sbuf_tensor("znorm", [P, 1], f32)
    svec = nc.alloc_sbuf_tensor("svec", [P, 1], f32)
    means = nc.alloc_sbuf_tensor("means", [P, D], f32)
    dtile = nc.alloc_sbuf_tensor("dtile", [P, D], f32)
    otile = nc.alloc_sbuf_tensor("otile", [P, D], f32)

    psum_s = nc.alloc_psum_tensor("psum_s", [P, D], f32)
    psum_c = nc.alloc_psum_tensor("psum_c", [P, 1], f32)

    lsems = [nc.alloc_semaphore(f"lsem{b}") for b in range(NB)]
    rsem = nc.alloc_semaphore("rsem")    # reduce done (per tile)
    osem = nc.alloc_semaphore("osem")    # one-hot built (per tile)
    msem = nc.alloc_semaphore("msem")    # matmuls done (2 per tile)
    psem = nc.alloc_semaphore("psem")    # preamble loads (3 x 16)
    bsem = nc.alloc_semaphore("bsem")    # blend done
    fsem = nc.alloc_semaphore("fsem")    # out store done
    vch = nc.alloc_semaphore("vch")      # blend chain

    def pstride(t):
        return t.ap().ap[0][0]

    def feat_tile_ap(j):
        return bass.AP(feat, j * P * FW, [[FW, P], [1, FW]])

    def ftile_red_ap(b):
        t = ftiles[b]
        return bass.AP(t, 0, [[pstride(t), P], [1, D], [D, GTOK]])

    def col(t, c, w=1):
        return bass.AP(t, c, [[pstride(t), P], [1, w]])

    def issue_loads(eng, parity):
        for j in range(T):
            if j % 2 != parity:
                continue
            b = j % NB
            if j >= NB:
                eng.wait_ge(rsem, j - NB + 1)
            eng.dma_start(ftiles[b].ap(), feat_tile_ap(j)).then_inc(lsems[b], 16)

    with nc.Block() as block:

        @block.scalar
        def _(scalar):
            scalar.dma_start(iota_sb.ap(), iota.ap()).then_inc(psem, 16)
            scalar.dma_start(meta_sb.ap(), meta.ap()).then_inc(psem, 16)
            scalar.dma_start(proto_sb.ap(), proto.ap()).then_inc(psem, 16)

        @block.sync
        def _(sync):
            issue_loads(sync, 0)
            sync.wait_ge(bsem, 1)
            sync.dma_start(out.ap(), otile.ap()).then_inc(fsem, 16)
            sync.wait_ge(fsem, 16)

        @block.gpsimd
        def _(gpsimd):
            issue_loads(gpsimd, 1)

        @block.vector
        def _(vector):
            vector.wait_ge(psem, 48)
            for j in range(T):
                b = j % NB
                vector.wait_ge(lsems[b], 16 * (j // NB + 1))
                if j >= 2:
                    # rhs/oh buffer j%2 free once tile j-2's matmuls retired
                    vector.wait_ge(msem, 2 * (j - 1))
                vector.tensor_reduce(
                    rhs[j % 2].ap(), ftile_red_ap(b),
                    axis=mybir.AxisListType.X, op=mybir.AluOpType.add,
                ).then_inc(rsem, 1)
                vector.tensor_scalar(
                    oh[j % 2].ap(), iota_sb.ap(), col(meta_sb, 2 * j), None,
                    mybir.AluOpType.is_equal,
                ).then_inc(osem, 1)

            # ---- blend ----
            vector.wait_ge(msem, 2 * T)
            vc = [0]

            def chain(ins):
                ins.then_inc(vch, 1)
                vc[0] += 1
                vector.wait_ge(vch, vc[0])

            chain(vector.tensor_copy(cnt.ap(), psum_c.ap()))
            chain(vector.tensor_scalar_max(rcp.ap(), cnt.ap(), 1.0))
            chain(vector.reciprocal(rcp.ap(), rcp.ap()))
            chain(vector.tensor_scalar(pres.ap(), cnt.ap(), 0.0, None,
                                       mybir.AluOpType.is_gt))
            if step_gt_warmup:
                chain(vector.tensor_reduce(
                    znorm.ap(), proto_sb.ap(),
                    axis=mybir.AxisListType.X, op=mybir.AluOpType.max,
                    apply_absolute_value=True,
                ))
                # svec = (znorm==0)*(1-LAM ... ) -> use_new flag
                chain(vector.tensor_scalar(svec.ap(), znorm.ap(), 0.0, None,
                                           mybir.AluOpType.is_equal))
            else:
                chain(vector.memset(svec.ap(), 1.0))
            # svec = pres * (0.1 + 0.9*use_new)
            chain(vector.tensor_scalar(svec.ap(), svec.ap(), LAM, 1.0 - LAM,
                                       mybir.AluOpType.mult,
                                       mybir.AluOpType.add))
            chain(vector.tensor_tensor(svec.ap(), svec.ap(), pres.ap(),
                                       mybir.AluOpType.mult))
            chain(vector.tensor_scalar_mul(means.ap(), psum_s.ap(), col(rcp, 0)))
            chain(vector.tensor_tensor(dtile.ap(), means.ap(), proto_sb.ap(),
                                       mybir.AluOpType.subtract))
            vector.scalar_tensor_tensor(
                otile.ap(), dtile.ap(), col(svec, 0), proto_sb.ap(),
                mybir.AluOpType.mult, mybir.AluOpType.add,
            ).then_inc(bsem, 1)

        @block.tensor
        def _(tensor):
            tensor.wait_ge(psem, 48)
            for j in range(T):
                tensor.wait_ge(rsem, j + 1)
                tensor.wait_ge(osem, j + 1)
                tensor.matmul(
                    psum_s.ap(), oh[j % 2].ap(), rhs[j % 2].ap(),
                    start=(j == 0), stop=(j == T - 1),
                ).then_inc(msem, 1)
                tensor.matmul(
                    psum_c.ap(), oh[j % 2].ap(), col(meta_sb, 2 * j + 1),
                    start=(j == 0), stop=(j == T - 1),
                ).then_inc(msem, 1)

    nc.compile()
    return nc


def shard_inputs(feature, label, prototype):
    """Returns (in_maps, cls_lists, T)."""
    n = feature.shape[0]
    counts = np.bincount(label, minlength=C)

    # greedy: biggest class -> least-loaded core (cap 128 classes/core)
    order_cls = np.argsort(-counts, kind="stable")
    core_load = np.zeros(N_CORES, dtype=np.int64)
    core_ncls = np.zeros(N_CORES, dtype=np.int64)
    cls_lists = [[] for _ in range(N_CORES)]
    nblk = (counts + GTOK - 1) // GTOK  # blocks per class
    for c in order_cls:
        k = min((k for k in range(N_CORES) if core_ncls[k] < P),
                key=lambda k: core_load[k])
        cls_lists[k].append(c)
        core_load[k] += nblk[c]
        core_ncls[k] += 1

    T = int(max(1, -(-core_load.max() // P)))
    cap_blk = T * P
    cap_tok = cap_blk * GTOK

    sort_order = np.argsort(label, kind="stable")
    starts = np.zeros(C + 1, dtype=np.int64)
    np.cumsum(counts, out=starts[1:])

    fdt = _feat_np_dtype()
    feature = np.ascontiguousarray(feature, dtype=np.float32)
    if FEAT_MODE != "f32":
        feature = feature.astype(fdt)

    src_all = np.full(N_CORES * cap_tok, -1, dtype=np.int64)
    in_maps = []
    iota_arr = np.tile(np.arange(P, dtype=np.float32), (P, 1))
    metas = []
    for k in range(N_CORES):
        base = k * cap_tok
        pos = 0
        meta = np.zeros((2, cap_blk), dtype=np.float32)  # [cls|cnt] per block
        blk = 0
        for li, c in enumerate(cls_lists[k]):
            ncv = int(counts[c])
            if ncv:
                src_all[base + pos: base + pos + ncv] = \
                    sort_order[starts[c]: starts[c] + ncv]
            nb = int(nblk[c])
            if nb:
                meta[0, blk: blk + nb] = li
                meta[1, blk: blk + nb] = GTOK
                meta[1, blk + nb - 1] = ncv - (nb - 1) * GTOK
            pos += nb * GTOK
            blk += nb
        metas.append(meta)

    packed = np.zeros((N_CORES * cap_tok, D), dtype=fdt)
    valid = src_all >= 0
    packed[valid] = feature[src_all[valid]]
    packed = packed.reshape(N_CORES, T * P, GTOK * D)

    proto32 = np.ascontiguousarray(prototype, dtype=np.float32)
    for k in range(N_CORES):
        cl = np.asarray(cls_lists[k], dtype=np.int64)
        pk = np.zeros((P, D), dtype=np.float32)
        pk[: len(cl)] = proto32[cl]
        # meta laid out [128, 2T]: block b=(j,p) -> cols (2j, 2j+1) at row p
        m = metas[k].reshape(2, T, P)  # [kind, tile, part]
        meta_k = np.ascontiguousarray(
            np.transpose(m, (2, 1, 0)).reshape(P, 2 * T))
        in_maps.append({
            "feature": np.ascontiguousarray(packed[k]),
            "blk_meta": meta_k,
            "prototype": pk,
            "iota": iota_arr,
        })
    return in_maps, cls_lists, T


_NC_CACHE = {}


def run(inputs: dict, trace: bool = False):
    feature = np.asarray(inputs["feature"])
    label = np.asarray(inputs["label"], dtype=np.int64)
    prototype = np.asarray(inputs["prototype"])
    step = int(np.asarray(inputs["step"]))

    in_maps, cls_lists, T = shard_inputs(feature, label, prototype)
    key = (T, step > WARMUP_STEP, FEAT_MODE)
    if key not in _NC_CACHE:
        _NC_CACHE[key] = build_nc(T, step > WARMUP_STEP)
    nc = _NC_CACHE[key]
    res = bass_utils.run_bass_kernel_spmd(
        nc, in_maps, core_ids=list(range(N_CORES)), trace=trace,
    )
    out = np.ascontiguousarray(prototype, dtype=np.float32).copy()
    for k in range(N_CORES):
        cl = np.asarray(cls_lists[k], dtype=np.int64)
        ok = np.asarray(res.results[k]["out"], dtype=np.float32)
        out[cl] = ok[: len(cl)]
    return out, res


def kernel(**inputs) -> np.ndarray:
    out, _ = run(inputs, trace=False)
    return out


# revision 5
# speedup vs baseline: 9.7862x; 1.8300x over previous
"""Trainium2 Bass kernel for the prototype-bank scatter-mean EMA update
(nn_Bank): class-sharded sorted-segment reduction across 8 NeuronCores.

Host (index/layout work only; all FP reduction arithmetic is on device):
  * argsort labels; assign each class to one core (greedy token balance,
    <=128 classes/core); concatenate each core's class segments, zero-
    padding every class to a multiple of GTOK tokens so each GTOK-token
    "block" is single-class.
  * feature blocks are packed feature-major [65, GTOK] in bf16: rows
    0..63 = the block's GTOK token features (transposed), row 64 = a
    1.0/0.0 valid-token indicator (so the same reduction that produces
    block feature sums also produces block counts).
  * per-core metadata: block -> local-class id; prototype rows for the
    core's classes; an iota row table.

Device, per core (T tiles; tile = 128 blocks = 128*GTOK tokens):
  1. Stream feature tiles [128, 65*GTOK] bf16 HBM->SBUF (sync queue).
  2. Block sums: tiles alternate between DVE tensor_reduce (true sums)
     and Pool avg-pool (sums/GTOK) -> rhs [128, 65] bf16.
  3. DVE builds one-hot lhsT [128 blocks, 128 local classes] bf16 via
     (iota == cls) * scale, scale = GTOK for Pool tiles (undoes the avg)
     and 1 for DVE tiles.
  4. PE: psum[cls, 65] += oh^T @ rhs, PSUM-accumulated over all T tiles
     -> per-class feature sums (cols 0..63) and counts (col 64).
  5. Blend: means = sums/max(cnt,1); out = proto + s*(means-proto) with
     s = present * (0.1 + 0.9*use_new)  [step>warmup branch].
  6. DMA out [128, D]; host scatters per-core rows back to [1000, 64].

No collective: every class is fully owned by one core.
"""

import numpy as np

import concourse.bacc as bacc
import concourse.bass as bass
import concourse.mybir as mybir
from concourse import bass_utils

C = 1000
D = 64
E = D + 1            # feature dims + count indicator
P = 128
GTOK = 32            # tokens per block (class padding granularity)
LAM = 0.9
WARMUP_STEP = 1000
N_CORES = 8
NB = 6               # feature tile buffers
NR = 4               # rhs / one-hot buffers
FW = E * GTOK        # free elems per feature tile partition


def tile_on_dve(j: int) -> bool:
    # all reduces on DVE (GpSimd elementwise runs at 0.42 eff — not worth it)
    return True


def build_nc(T: int, step_gt_warmup: bool):
    f32 = mybir.dt.float32
    bf16 = mybir.dt.bfloat16

    dcount = [0] * (T + 1)  # dcount[j+1] = #DVE tiles among 0..j
    pcount = [0] * (T + 1)
    for j in range(T):
        dcount[j + 1] = dcount[j] + (1 if tile_on_dve(j) else 0)
        pcount[j + 1] = pcount[j] + (0 if tile_on_dve(j) else 1)

    nc = bacc.Bacc("TRN2", target_bir_lowering=False, debug=False,
                   num_devices=N_CORES)

    feat = nc.dram_tensor("feature", [T * P, FW], bf16, kind="ExternalInput")
    meta = nc.dram_tensor("blk_meta", [P, T], f32, kind="ExternalInput")
    proto = nc.dram_tensor("prototype", [P, D], f32, kind="ExternalInput")
    iota = nc.dram_tensor("iota", [P, P], bf16, kind="ExternalInput")
    out = nc.dram_tensor("out", [P, D], f32, kind="ExternalOutput")

    ftiles = [nc.alloc_sbuf_tensor(f"ftile{b}", [P, FW], bf16) for b in range(NB)]
    iota_sb = nc.alloc_sbuf_tensor("iota_sb", [P, P], bf16)
    meta_sb = nc.alloc_sbuf_tensor("meta_sb", [P, T], f32)
    proto_sb = nc.alloc_sbuf_tensor("proto_sb", [P, D], f32)
    rhs = [nc.alloc_sbuf_tensor(f"rhs{i}", [P, E], bf16) for i in range(NR)]
    oh = [nc.alloc_sbuf_tensor(f"oh{i}", [P, P], bf16) for i in range(NR)]
    cnt = nc.alloc_sbuf_tensor("cnt", [P, 1], f32)
    rcp = nc.alloc_sbuf_tensor("rcp", [P, 1], f32)
    pres = nc.alloc_sbuf_tensor("pres", [P, 1], f32)
    znorm = nc.alloc_sbuf_tensor("znorm", [P, 1], f32)
    svec = nc.alloc_sbuf_tensor("svec", [P, 1], f32)
    means = nc.alloc_sbuf_tensor("means", [P, D], f32)
    dtile = nc.alloc_sbuf_tensor("dtile", [P, D], f32)
    otile = nc.alloc_sbuf_tensor("otile", [P, D], f32)

    psum_s = nc.alloc_psum_tensor("psum_s", [P, E], f32)

    lsems = [nc.alloc_semaphore(f"lsem{b}") for b in range(NB)]
    rsem_d = nc.alloc_semaphore("rsem_d")  # DVE reduces done
    rsem_p = nc.alloc_semaphore("rsem_p")  # Pool reduces done
    ohsem = nc.alloc_semaphore("ohsem")    # one-hots built
    msem = nc.alloc_semaphore("msem")      # matmuls done (1 per tile)
    psem = nc.alloc_semaphore("psem")      # preamble loads (3 x 16)
    bsem = nc.alloc_semaphore("bsem")      # blend done
    fsem = nc.alloc_semaphore("fsem")      # out store done
    vch = nc.alloc_semaphore("vch")        # blend chain

    def pstride(t):
        return t.ap().ap[0][0]

    def feat_tile_ap(j):
        return bass.AP(feat, j * P * FW, [[FW, P], [1, FW]])

    def ftile_red_ap(b):
        t = ftiles[b]
        return bass.AP(t, 0, [[pstride(t), P], [GTOK, E], [1, GTOK]])

    def col(t, c, w=1):
        return bass.AP(t, c, [[pstride(t), P], [1, w]])

    def wait_reduced(eng, j):
        """Wait until tile j's reduce has retired."""
        if tile_on_dve(j):
            eng.wait_ge(rsem_d, dcount[j + 1])
        else:
            eng.wait_ge(rsem_p, pcount[j + 1])

    with nc.allow_low_precision("bf16 block sums; exact count col"), \
            nc.Block() as block:

        @block.scalar
        def _(scalar):
            scalar.dma_start(iota_sb.ap(), iota.ap()).then_inc(psem, 16)
            scalar.dma_start(meta_sb.ap(), meta.ap()).then_inc(psem, 16)
            scalar.dma_start(proto_sb.ap(), proto.ap()).then_inc(psem, 16)

        @block.sync
        def _(sync):
            for j in range(T):
                b = j % NB
                if j >= NB:
                    wait_reduced(sync, j - NB)
                sync.dma_start(ftiles[b].ap(), feat_tile_ap(j)).then_inc(lsems[b], 16)
            sync.wait_ge(bsem, 1)
            sync.dma_start(out.ap(), otile.ap()).then_inc(fsem, 16)
            sync.wait_ge(fsem, 16)

        @block.gpsimd
        def _(gpsimd):
            for j in range(T):
                if tile_on_dve(j):
                    continue
                b = j % NB
                gpsimd.wait_ge(lsems[b], 16 * (j // NB + 1))
                if j >= NR:
                    gpsimd.wait_ge(msem, j - NR + 1)
                gpsimd.pool_avg(rhs[j % NR].ap(), ftile_red_ap(b)) \
                    .then_inc(rsem_p, 1)

        @block.vector
        def _(vector):
            vector.wait_ge(psem, 48)
            for j in range(T):
                b = j % NB
                if j >= NR:
                    vector.wait_ge(msem, j - NR + 1)
                if tile_on_dve(j):
                    vector.wait_ge(lsems[b], 16 * (j // NB + 1))
                    vector.tensor_reduce(
                        rhs[j % NR].ap(), ftile_red_ap(b),
                        axis=mybir.AxisListType.X, op=mybir.AluOpType.add,
                    ).then_inc(rsem_d, 1)
                vector.tensor_scalar(
                    oh[j % NR].ap(), iota_sb.ap(), col(meta_sb, j),
                    1.0 if tile_on_dve(j) else float(GTOK),
                    mybir.AluOpType.is_equal, mybir.AluOpType.mult,
                ).then_inc(ohsem, 1)

            # ---- blend ----
            vector.wait_ge(msem, T)
            vc = [0]

            def chain(ins):
                ins.then_inc(vch, 1)
                vc[0] += 1
                vector.wait_ge(vch, vc[0])

            chain(vector.tensor_copy(cnt.ap(), col(psum_s, D)))
            chain(vector.tensor_scalar_max(rcp.ap(), cnt.ap(), 1.0))
            chain(vector.reciprocal(rcp.ap(), rcp.ap()))
            chain(vector.tensor_scalar(pres.ap(), cnt.ap(), 0.5, None,
                                       mybir.AluOpType.is_gt))
            if step_gt_warmup:
                chain(vector.tensor_reduce(
                    znorm.ap(), proto_sb.ap(),
                    axis=mybir.AxisListType.X, op=mybir.AluOpType.max,
                    apply_absolute_value=True,
                ))
                chain(vector.tensor_scalar(svec.ap(), znorm.ap(), 0.0, None,
                                           mybir.AluOpType.is_equal))
            else:
                chain(vector.memset(svec.ap(), 1.0))
            # svec = pres * (0.1 + 0.9*use_new)
            chain(vector.tensor_scalar(svec.ap(), svec.ap(), LAM, 1.0 - LAM,
                                       mybir.AluOpType.mult,
                                       mybir.AluOpType.add))
            chain(vector.tensor_tensor(svec.ap(), svec.ap(), pres.ap(),
                                       mybir.AluOpType.mult))
            chain(vector.tensor_scalar_mul(
                means.ap(), bass.AP(psum_s, 0, [[pstride(psum_s), P], [1, D]]),
                col(rcp, 0)))
            chain(vector.tensor_tensor(dtile.ap(), means.ap(), proto_sb.ap(),
                                       mybir.AluOpType.subtract))
            vector.scalar_tensor_tensor(
                otile.ap(), dtile.ap(), col(svec, 0), proto_sb.ap(),
                mybir.AluOpType.mult, mybir.AluOpType.add,
            ).then_inc(bsem, 1)

        @block.tensor
        def _(tensor):
            for j in range(T):
                wait_reduced(tensor, j)
                tensor.wait_ge(ohsem, j + 1)
                tensor.matmul(
                    psum_s.ap(), oh[j % NR].ap(), rhs[j % NR].ap(),
                    start=(j == 0), stop=(j == T - 1),
                ).then_inc(msem, 1)

    nc.compile()
    return nc


def shard_inputs(feature, label, prototype):
    """Returns (in_maps, cls_lists, T)."""
    import ml_dtypes
    bf16 = ml_dtypes.bfloat16

    counts = np.bincount(label, minlength=C)

    # greedy: biggest class -> least-loaded core (cap 128 classes/core)
    order_cls = np.argsort(-counts, kind="stable")
    core_load = np.zeros(N_CORES, dtype=np.int64)
    core_ncls = np.zeros(N_CORES, dtype=np.int64)
    cls_lists = [[] for _ in range(N_CORES)]
    nblk = (counts + GTOK - 1) // GTOK  # blocks per class
    for c in order_cls:
        k = min((k for k in range(N_CORES) if core_ncls[k] < P),
                key=lambda k: core_load[k])
        cls_lists[k].append(c)
        core_load[k] += nblk[c]
        core_ncls[k] += 1

    T = int(max(1, -(-core_load.max() // P)))
    cap_blk = T * P
    cap_tok = cap_blk * GTOK

    sort_order = np.argsort(label, kind="stable")
    starts = np.zeros(C + 1, dtype=np.int64)
    np.cumsum(counts, out=starts[1:])

    feat_bf = np.ascontiguousarray(feature, dtype=np.float32).astype(bf16)

    src_all = np.full(N_CORES * cap_tok, -1, dtype=np.int64)
    metas = []
    for k in range(N_CORES):
        base = k * cap_tok
        pos = 0
        mcls = np.zeros(cap_blk, dtype=np.float32)
        blk = 0
        for li, c in enumerate(cls_lists[k]):
            ncv = int(counts[c])
            if ncv:
                src_all[base + pos: base + pos + ncv] = \
                    sort_order[starts[c]: starts[c] + ncv]
            nb = int(nblk[c])
            if nb:
                mcls[blk: blk + nb] = li
            pos += nb * GTOK
            blk += nb
        metas.append(mcls)

    nblk_tot = N_CORES * cap_blk
    valid = src_all >= 0
    tok = np.zeros((nblk_tot * GTOK, D), dtype=bf16)
    tok[valid] = feat_bf[src_all[valid]]
    arr = np.empty((nblk_tot, E, GTOK), dtype=bf16)
    arr[:, :D, :] = tok.reshape(nblk_tot, GTOK, D).swapaxes(1, 2)
    arr[:, D, :] = valid.reshape(nblk_tot, GTOK).astype(bf16)
    arr = arr.reshape(N_CORES, T * P, FW)

    proto32 = np.ascontiguousarray(prototype, dtype=np.float32)
    iota_arr = np.tile(np.arange(P, dtype=np.float32), (P, 1)).astype(bf16)
    in_maps = []
    for k in range(N_CORES):
        cl = np.asarray(cls_lists[k], dtype=np.int64)
        pk = np.zeros((P, D), dtype=np.float32)
        pk[: len(cl)] = proto32[cl]
        # block b=(tile j, partition p) -> meta[p, j]
        meta_k = np.ascontiguousarray(
            metas[k].reshape(T, P).T)
        in_maps.append({
            "feature": np.ascontiguousarray(arr[k]),
            "blk_meta": meta_k,
            "prototype": pk,
            "iota": iota_arr,
        })
    return in_maps, cls_lists, T


_NC_CACHE = {}


def run(inputs: dict, trace: bool = False):
    feature = np.asarray(inputs["feature"])
    label = np.asarray(inputs["label"], dtype=np.int64)
    prototype = np.asarray(inputs["prototype"])
    step = int(np.asarray(inputs["step"]))

    in_maps, cls_lists, T = shard_inputs(feature, label, prototype)
    key = (T, step > WARMUP_STEP)
    if key not in _NC_CACHE:
        _NC_CACHE[key] = build_nc(T, step > WARMUP_STEP)
    nc = _NC_CACHE[key]
    res = bass_utils.run_bass_kernel_spmd(
        nc, in_maps, core_ids=list(range(N_CORES)), trace=trace,
    )
    out = np.ascontiguousarray(prototype, dtype=np.float32).copy()
    for k in range(N_CORES):
        cl = np.asarray(cls_lists[k], dtype=np.int64)
        ok = np.asarray(res.results[k]["out"], dtype=np.float32)
        out[cl] = ok[: len(cl)]
    return out, res


def kernel(**inputs) -> np.ndarray:
    out, _ = run(inputs, trace=False)
    return out


# revision 10
# speedup vs baseline: 10.4971x; 1.0726x over previous
"""Trainium2 Bass kernel for the prototype-bank scatter-mean EMA update
(nn_Bank): class-sharded sorted-segment reduction across 8 NeuronCores.

Host (index/layout work only; all FP reduction arithmetic is on device):
  * argsort labels; assign each class to one core (greedy token balance,
    <=128 classes/core); concatenate each core's class segments, zero-
    padding every class to a multiple of GTOK tokens so each GTOK-token
    "block" is single-class.
  * feature blocks are packed feature-major [65, GTOK] in bf16: rows
    0..63 = the block's GTOK token features (transposed), row 64 = a
    1.0/0.0 valid-token indicator (so the same reduction that produces
    block feature sums also produces block counts).
  * per-core metadata: block -> local-class id; prototype rows for the
    core's classes; an iota row table.

Device, per core (T tiles; tile = 128 blocks = 128*GTOK tokens):
  1. Stream feature tiles [128, 65*GTOK] bf16 HBM->SBUF (sync queue).
  2. Block sums: tiles alternate between DVE tensor_reduce (true sums)
     and Pool avg-pool (sums/GTOK) -> rhs [128, 65] bf16.
  3. DVE builds one-hot lhsT [128 blocks, 128 local classes] bf16 via
     (iota == cls) * scale, scale = GTOK for Pool tiles (undoes the avg)
     and 1 for DVE tiles.
  4. PE: psum[cls, 65] += oh^T @ rhs, PSUM-accumulated over all T tiles
     -> per-class feature sums (cols 0..63) and counts (col 64).
  5. Blend: means = sums/max(cnt,1); out = proto + s*(means-proto) with
     s = present * (0.1 + 0.9*use_new)  [step>warmup branch].
  6. DMA out [128, D]; host scatters per-core rows back to [1000, 64].

No collective: every class is fully owned by one core.
"""

import numpy as np

import concourse.bacc as bacc
import concourse.bass as bass
import concourse.mybir as mybir
from concourse import bass_utils

C = 1000
D = 64
E = D + 1            # feature dims + count indicator
P = 128
GTOK = 32            # tokens per block (class padding granularity)
LAM = 0.9
WARMUP_STEP = 1000
N_CORES = 8
NB = 8               # feature tile buffers
NR = 4               # rhs / one-hot buffers
FW = E * GTOK        # free elems per feature tile partition


def tile_on_dve(j: int) -> bool:
    # ~3:1 DVE:Pool split — bf16 tree-add runs ~1.5us/tile on DVE (2x_1p
    # packed mode) vs ~4.5us/tile on Pool (0.42 sw efficiency)
    return j % 4 != 3


def build_nc(T: int, step_gt_warmup: bool):
    f32 = mybir.dt.float32
    bf16 = mybir.dt.bfloat16

    dcount = [0] * (T + 1)  # dcount[j+1] = #DVE tiles among 0..j
    pcount = [0] * (T + 1)
    for j in range(T):
        dcount[j + 1] = dcount[j] + (1 if tile_on_dve(j) else 0)
        pcount[j + 1] = pcount[j] + (0 if tile_on_dve(j) else 1)

    nc = bacc.Bacc("TRN2", target_bir_lowering=False, debug=False,
                   num_devices=N_CORES)

    feat = nc.dram_tensor("feature", [T * P, FW], bf16, kind="ExternalInput")
    meta = nc.dram_tensor("blk_meta", [P, T], f32, kind="ExternalInput")
    proto = nc.dram_tensor("prototype", [P, D], f32, kind="ExternalInput")
    iota = nc.dram_tensor("iota", [P, P], bf16, kind="ExternalInput")
    out = nc.dram_tensor("out", [P, D], f32, kind="ExternalOutput")

    ftiles = [nc.alloc_sbuf_tensor(f"ftile{b}", [P, FW], bf16) for b in range(NB)]
    iota_sb = nc.alloc_sbuf_tensor("iota_sb", [P, P], bf16)
    meta_sb = nc.alloc_sbuf_tensor("meta_sb", [P, T], f32)
    proto_sb = nc.alloc_sbuf_tensor("proto_sb", [P, D], f32)
    rhs = [nc.alloc_sbuf_tensor(f"rhs{i}", [P, E], bf16) for i in range(NR)]
    oh = [nc.alloc_sbuf_tensor(f"oh{i}", [P, P], bf16) for i in range(NR)]
    cnt = nc.alloc_sbuf_tensor("cnt", [P, 1], f32)
    rcp = nc.alloc_sbuf_tensor("rcp", [P, 1], f32)
    pres = nc.alloc_sbuf_tensor("pres", [P, 1], f32)
    znorm = nc.alloc_sbuf_tensor("znorm", [P, 1], f32)
    svec = nc.alloc_sbuf_tensor("svec", [P, 1], f32)
    means = nc.alloc_sbuf_tensor("means", [P, D], f32)
    dtile = nc.alloc_sbuf_tensor("dtile", [P, D], f32)
    otile = nc.alloc_sbuf_tensor("otile", [P, D], f32)

    psum_s = nc.alloc_psum_tensor("psum_s", [P, E], f32)

    lsems = [nc.alloc_semaphore(f"lsem{b}") for b in range(NB)]
    rsem_d = nc.alloc_semaphore("rsem_d")  # DVE reduces done
    rsem_p = nc.alloc_semaphore("rsem_p")  # Pool reduces done
    ohsem = nc.alloc_semaphore("ohsem")    # one-hots built
    msem = nc.alloc_semaphore("msem")      # matmuls done (1 per tile)
    psem = nc.alloc_semaphore("psem")      # preamble loads (3 x 16)
    bsem = nc.alloc_semaphore("bsem")      # blend done
    fsem = nc.alloc_semaphore("fsem")      # out store done
    vch = nc.alloc_semaphore("vch")        # blend chain

    def pstride(t):
        return t.ap().ap[0][0]

    def feat_tile_ap(j):
        return bass.AP(feat, j * P * FW, [[FW, P], [1, FW]])

    def ftile_red_ap(b):
        t = ftiles[b]
        return bass.AP(t, 0, [[pstride(t), P], [GTOK, E], [1, GTOK]])

    def col(t, c, w=1):
        return bass.AP(t, c, [[pstride(t), P], [1, w]])

    def wait_reduced(eng, j):
        """Wait until tile j's reduce has retired."""
        if tile_on_dve(j):
            eng.wait_ge(rsem_d, dcount[j + 1])
        else:
            eng.wait_ge(rsem_p, pcount[j + 1])

    def tree_reduce(eng, j, rsem):
        """In-place bf16 pairwise tree: ftile[b] [P, E, GTOK] summed over
        GTOK -> rhs[j%NR] [P, E]. Consumes (overwrites) the ftile."""
        b = j % NB
        t = ftiles[b]
        eng.wait_ge(lsems[b], 16 * (j // NB + 1))
        w = GTOK // 2
        while w >= 2:
            eng.tensor_tensor(
                bass.AP(t, 0, [[pstride(t), P], [GTOK, E], [1, w]]),
                bass.AP(t, 0, [[pstride(t), P], [GTOK, E], [1, w]]),
                bass.AP(t, w, [[pstride(t), P], [GTOK, E], [1, w]]),
                mybir.AluOpType.add,
            )
            w //= 2
        if j >= NR:
            eng.wait_ge(msem, j - NR + 1)  # rhs slot freed by matmul j-NR
        eng.tensor_tensor(
            rhs[j % NR].ap(),
            bass.AP(t, 0, [[pstride(t), P], [GTOK, E]]),
            bass.AP(t, 1, [[pstride(t), P], [GTOK, E]]),
            mybir.AluOpType.add,
        ).then_inc(rsem, 1)

    with nc.allow_low_precision("bf16 block sums; exact count col"), \
            nc.Block() as block:

        @block.scalar
        def _(scalar):
            scalar.dma_start(iota_sb.ap(), iota.ap()).then_inc(psem, 16)
            scalar.dma_start(meta_sb.ap(), meta.ap()).then_inc(psem, 16)
            scalar.dma_start(proto_sb.ap(), proto.ap()).then_inc(psem, 16)

        @block.sync
        def _(sync):
            for j in range(T):
                b = j % NB
                if j >= NB:
                    wait_reduced(sync, j - NB)
                sync.dma_start(ftiles[b].ap(), feat_tile_ap(j)).then_inc(lsems[b], 16)
            sync.wait_ge(bsem, 1)
            sync.dma_start(out.ap(), otile.ap()).then_inc(fsem, 16)
            sync.wait_ge(fsem, 16)

        @block.gpsimd
        def _(gpsimd):
            for j in range(T):
                if not tile_on_dve(j):
                    tree_reduce(gpsimd, j, rsem_p)

        @block.vector
        def _(vector):
            vector.wait_ge(psem, 48)
            for j in range(T):
                if tile_on_dve(j):
                    tree_reduce(vector, j, rsem_d)
                elif j >= NR:
                    vector.wait_ge(msem, j - NR + 1)  # oh slot reuse
                vector.tensor_scalar(
                    oh[j % NR].ap(), iota_sb.ap(), col(meta_sb, j), 1.0,
                    mybir.AluOpType.is_equal, mybir.AluOpType.mult,
                ).then_inc(ohsem, 1)

            # ---- blend ----
            vector.wait_ge(msem, T)
            vc = [0]

            def chain(ins):
                ins.then_inc(vch, 1)
                vc[0] += 1
                vector.wait_ge(vch, vc[0])

            chain(vector.tensor_copy(cnt.ap(), col(psum_s, D)))
            chain(vector.tensor_scalar_max(rcp.ap(), cnt.ap(), 1.0))
            chain(vector.reciprocal(rcp.ap(), rcp.ap()))
            chain(vector.tensor_scalar(pres.ap(), cnt.ap(), 0.5, None,
                                       mybir.AluOpType.is_gt))
            if step_gt_warmup:
                chain(vector.tensor_reduce(
                    znorm.ap(), proto_sb.ap(),
                    axis=mybir.AxisListType.X, op=mybir.AluOpType.max,
                    apply_absolute_value=True,
                ))
                chain(vector.tensor_scalar(svec.ap(), znorm.ap(), 0.0, None,
                                           mybir.AluOpType.is_equal))
            else:
                chain(vector.memset(svec.ap(), 1.0))
            # svec = pres * (0.1 + 0.9*use_new)
            chain(vector.tensor_scalar(svec.ap(), svec.ap(), LAM, 1.0 - LAM,
                                       mybir.AluOpType.mult,
                                       mybir.AluOpType.add))
            chain(vector.tensor_tensor(svec.ap(), svec.ap(), pres.ap(),
                                       mybir.AluOpType.mult))
            chain(vector.tensor_scalar_mul(
                means.ap(), bass.AP(psum_s, 0, [[pstride(psum_s), P], [1, D]]),
                col(rcp, 0)))
            chain(vector.tensor_tensor(dtile.ap(), means.ap(), proto_sb.ap(),
                                       mybir.AluOpType.subtract))
            vector.scalar_tensor_tensor(
                otile.ap(), dtile.ap(), col(svec, 0), proto_sb.ap(),
                mybir.AluOpType.mult, mybir.AluOpType.add,
            ).then_inc(bsem, 1)

        @block.tensor
        def _(tensor):
            for j in range(T):
                wait_reduced(tensor, j)
                tensor.wait_ge(ohsem, j + 1)
                tensor.matmul(
                    psum_s.ap(), oh[j % NR].ap(), rhs[j % NR].ap(),
                    start=(j == 0), stop=(j == T - 1),
                ).then_inc(msem, 1)

    nc.compile()
    return nc


def shard_inputs(feature, label, prototype):
    """Returns (in_maps, cls_lists, T)."""
    import ml_dtypes
    bf16 = ml_dtypes.bfloat16

    counts = np.bincount(label, minlength=C)

    # greedy: biggest class -> least-loaded core (cap 128 classes/core)
    order_cls = np.argsort(-counts, kind="stable")
    core_load = np.zeros(N_CORES, dtype=np.int64)
    core_ncls = np.zeros(N_CORES, dtype=np.int64)
    cls_lists = [[] for _ in range(N_CORES)]
    nblk = (counts + GTOK - 1) // GTOK  # blocks per class
    for c in order_cls:
        k = min((k for k in range(N_CORES) if core_ncls[k] < P),
                key=lambda k: core_load[k])
        cls_lists[k].append(c)
        core_load[k] += nblk[c]
        core_ncls[k] += 1

    T = int(max(1, -(-core_load.max() // P)))
    cap_blk = T * P
    cap_tok = cap_blk * GTOK

    sort_order = np.argsort(label, kind="stable")
    starts = np.zeros(C + 1, dtype=np.int64)
    np.cumsum(counts, out=starts[1:])

    feat_bf = np.ascontiguousarray(feature, dtype=np.float32).astype(bf16)

    src_all = np.full(N_CORES * cap_tok, -1, dtype=np.int64)
    metas = []
    for k in range(N_CORES):
        base = k * cap_tok
        pos = 0
        mcls = np.zeros(cap_blk, dtype=np.float32)
        blk = 0
        for li, c in enumerate(cls_lists[k]):
            ncv = int(counts[c])
            if ncv:
                src_all[base + pos: base + pos + ncv] = \
                    sort_order[starts[c]: starts[c] + ncv]
            nb = int(nblk[c])
            if nb:
                mcls[blk: blk + nb] = li
            pos += nb * GTOK
            blk += nb
        metas.append(mcls)

    nblk_tot = N_CORES * cap_blk
    valid = src_all >= 0
    tok = np.zeros((nblk_tot * GTOK, D), dtype=bf16)
    tok[valid] = feat_bf[src_all[valid]]
    arr = np.empty((nblk_tot, E, GTOK), dtype=bf16)
    arr[:, :D, :] = tok.reshape(nblk_tot, GTOK, D).swapaxes(1, 2)
    arr[:, D, :] = valid.reshape(nblk_tot, GTOK).astype(bf16)
    arr = arr.reshape(N_CORES, T * P, FW)

    proto32 = np.ascontiguousarray(prototype, dtype=np.float32)
    iota_arr = np.tile(np.arange(P, dtype=np.float32), (P, 1)).astype(bf16)
    in_maps = []
    for k in range(N_CORES):
        cl = np.asarray(cls_lists[k], dtype=np.int64)
        pk = np.zeros((P, D), dtype=np.float32)
        pk[: len(cl)] = proto32[cl]
        # block b=(tile j, partition p) -> meta[p, j]
        meta_k = np.ascontiguousarray(
            metas[k].reshape(T, P).T)
        in_maps.append({
            "feature": np.ascontiguousarray(arr[k]),
            "blk_meta": meta_k,
            "prototype": pk,
            "iota": iota_arr,
        })
    return in_maps, cls_lists, T


_NC_CACHE = {}


def run(inputs: dict, trace: bool = False):
    feature = np.asarray(inputs["feature"])
    label = np.asarray(inputs["label"], dtype=np.int64)
    prototype = np.asarray(inputs["prototype"])
    step = int(np.asarray(inputs["step"]))

    in_maps, cls_lists, T = shard_inputs(feature, label, prototype)
    key = (T, step > WARMUP_STEP)
    if key not in _NC_CACHE:
        _NC_CACHE[key] = build_nc(T, step > WARMUP_STEP)
    nc = _NC_CACHE[key]
    res = bass_utils.run_bass_kernel_spmd(
        nc, in_maps, core_ids=list(range(N_CORES)), trace=trace,
    )
    out = np.ascontiguousarray(prototype, dtype=np.float32).copy()
    for k in range(N_CORES):
        cl = np.asarray(cls_lists[k], dtype=np.int64)
        ok = np.asarray(res.results[k]["out"], dtype=np.float32)
        out[cl] = ok[: len(cl)]
    return out, res


def kernel(**inputs) -> np.ndarray:
    out, _ = run(inputs, trace=False)
    return out


# revision 12
# speedup vs baseline: 11.3729x; 1.0834x over previous
"""Trainium2 Bass kernel for the prototype-bank scatter-mean EMA update
(nn_Bank): class-sharded sorted-segment reduction across 8 NeuronCores.

Host (index/layout work only; all FP reduction arithmetic is on device):
  * argsort labels; assign each class to one core (greedy token balance,
    <=128 classes/core); concatenate each core's class segments, zero-
    padding every class to a multiple of GTOK tokens so each GTOK-token
    "block" is single-class.
  * feature blocks are packed feature-major [65, GTOK] in bf16: rows
    0..63 = the block's GTOK token features (transposed), row 64 = a
    1.0/0.0 valid-token indicator (so the same reduction that produces
    block feature sums also produces block counts).
  * per-core metadata: block -> local-class id; prototype rows for the
    core's classes; an iota row table.

Device, per core (T tiles; tile = 128 blocks = 128*GTOK tokens):
  1. Stream feature tiles [128, 65*GTOK] bf16 HBM->SBUF (sync queue).
  2. Block sums: tiles alternate between DVE tensor_reduce (true sums)
     and Pool avg-pool (sums/GTOK) -> rhs [128, 65] bf16.
  3. DVE builds one-hot lhsT [128 blocks, 128 local classes] bf16 via
     (iota == cls) * scale, scale = GTOK for Pool tiles (undoes the avg)
     and 1 for DVE tiles.
  4. PE: psum[cls, 65] += oh^T @ rhs, PSUM-accumulated over all T tiles
     -> per-class feature sums (cols 0..63) and counts (col 64).
  5. Blend: means = sums/max(cnt,1); out = proto + s*(means-proto) with
     s = present * (0.1 + 0.9*use_new)  [step>warmup branch].
  6. DMA out [128, D]; host scatters per-core rows back to [1000, 64].

No collective: every class is fully owned by one core.
"""

import numpy as np

import concourse.bacc as bacc
import concourse.bass as bass
import concourse.mybir as mybir
from concourse import bass_utils

C = 1000
D = 64
E = D + 1            # feature dims + count indicator
P = 128
GTOK = 32            # tokens per block (class padding granularity)
LAM = 0.9
WARMUP_STEP = 1000
N_CORES = 8
NB = 8               # feature tile buffers
NR = 4               # rhs / one-hot buffers
FW = E * GTOK        # free elems per feature tile partition


OHB = 8              # one-hot batch size (tiles per build instruction)


def tile_on_dve(j: int) -> bool:
    # ~4:1 DVE:Pool split: contiguous bf16 tree-add ~1.5us/tile on DVE
    # (plus the one-hot batch builds) vs ~5-6.5us/tile on Pool
    return j % 5 != 4


def build_nc(T: int, step_gt_warmup: bool):
    f32 = mybir.dt.float32
    bf16 = mybir.dt.bfloat16

    dcount = [0] * (T + 1)  # dcount[j+1] = #DVE tiles among 0..j
    pcount = [0] * (T + 1)
    for j in range(T):
        dcount[j + 1] = dcount[j] + (1 if tile_on_dve(j) else 0)
        pcount[j + 1] = pcount[j] + (0 if tile_on_dve(j) else 1)

    nc = bacc.Bacc("TRN2", target_bir_lowering=False, debug=False,
                   num_devices=N_CORES)

    feat = nc.dram_tensor("feature", [T * P, FW], bf16, kind="ExternalInput")
    meta = nc.dram_tensor("blk_meta", [P, T], f32, kind="ExternalInput")
    proto = nc.dram_tensor("prototype", [P, D], f32, kind="ExternalInput")
    iota = nc.dram_tensor("iota", [P, P], f32, kind="ExternalInput")
    out = nc.dram_tensor("out", [P, D], f32, kind="ExternalOutput")

    ftiles = [nc.alloc_sbuf_tensor(f"ftile{b}", [P, FW], bf16) for b in range(NB)]
    iota_sb = nc.alloc_sbuf_tensor("iota_sb", [P, P], f32)
    meta_sb = nc.alloc_sbuf_tensor("meta_sb", [P, T], f32)
    proto_sb = nc.alloc_sbuf_tensor("proto_sb", [P, D], f32)
    rhs = [nc.alloc_sbuf_tensor(f"rhs{i}", [P, E], bf16) for i in range(NR)]
    oh = [nc.alloc_sbuf_tensor(f"oh{i}", [P, OHB * P], bf16) for i in range(2)]
    cnt = nc.alloc_sbuf_tensor("cnt", [P, 1], f32)
    rcp = nc.alloc_sbuf_tensor("rcp", [P, 1], f32)
    pres = nc.alloc_sbuf_tensor("pres", [P, 1], f32)
    znorm = nc.alloc_sbuf_tensor("znorm", [P, 1], f32)
    svec = nc.alloc_sbuf_tensor("svec", [P, 1], f32)
    means = nc.alloc_sbuf_tensor("means", [P, D], f32)
    dtile = nc.alloc_sbuf_tensor("dtile", [P, D], f32)
    otile = nc.alloc_sbuf_tensor("otile", [P, D], f32)

    psum_s = nc.alloc_psum_tensor("psum_s", [P, E], f32)

    lsems = [nc.alloc_semaphore(f"lsem{b}") for b in range(NB)]
    rsem_d = nc.alloc_semaphore("rsem_d")  # DVE reduces done
    rsem_p = nc.alloc_semaphore("rsem_p")  # Pool reduces done
    ohsem = nc.alloc_semaphore("ohsem")    # one-hots built
    msem = nc.alloc_semaphore("msem")      # matmuls done (1 per tile)
    psem = nc.alloc_semaphore("psem")      # preamble loads (3 x 16)
    bsem = nc.alloc_semaphore("bsem")      # blend done
    fsem = nc.alloc_semaphore("fsem")      # out store done
    vch = nc.alloc_semaphore("vch")        # blend chain

    def pstride(t):
        return t.ap().ap[0][0]

    def feat_tile_ap(j):
        return bass.AP(feat, j * P * FW, [[FW, P], [1, FW]])

    def ftile_red_ap(b):
        t = ftiles[b]
        return bass.AP(t, 0, [[pstride(t), P], [GTOK, E], [1, GTOK]])

    def col(t, c, w=1):
        return bass.AP(t, c, [[pstride(t), P], [1, w]])

    def wait_reduced(eng, j):
        """Wait until tile j's reduce has retired."""
        if tile_on_dve(j):
            eng.wait_ge(rsem_d, dcount[j + 1])
        else:
            eng.wait_ge(rsem_p, pcount[j + 1])

    def tree_reduce(eng, j, rsem):
        """In-place bf16 pairwise tree: ftile[b] [P, E, GTOK] summed over
        GTOK -> rhs[j%NR] [P, E]. Consumes (overwrites) the ftile."""
        b = j % NB
        t = ftiles[b]
        eng.wait_ge(lsems[b], 16 * (j // NB + 1))
        w = GTOK // 2
        while w >= 2:
            eng.tensor_tensor(
                bass.AP(t, 0, [[pstride(t), P], [1, w * E]]),
                bass.AP(t, 0, [[pstride(t), P], [1, w * E]]),
                bass.AP(t, w * E, [[pstride(t), P], [1, w * E]]),
                mybir.AluOpType.add,
            )
            w //= 2
        if j >= NR:
            eng.wait_ge(msem, j - NR + 1)  # rhs slot freed by matmul j-NR
        eng.tensor_tensor(
            rhs[j % NR].ap(),
            bass.AP(t, 0, [[pstride(t), P], [1, E]]),
            bass.AP(t, E, [[pstride(t), P], [1, E]]),
            mybir.AluOpType.add,
        ).then_inc(rsem, 1)

    with nc.allow_low_precision("bf16 block sums; exact count col"), \
            nc.Block() as block:

        @block.scalar
        def _(scalar):
            scalar.dma_start(iota_sb.ap(), iota.ap()).then_inc(psem, 16)
            scalar.dma_start(meta_sb.ap(), meta.ap()).then_inc(psem, 16)
            scalar.dma_start(proto_sb.ap(), proto.ap()).then_inc(psem, 16)

        @block.sync
        def _(sync):
            for j in range(T):
                b = j % NB
                if j >= NB:
                    wait_reduced(sync, j - NB)
                sync.dma_start(ftiles[b].ap(), feat_tile_ap(j)).then_inc(lsems[b], 16)
            sync.wait_ge(bsem, 1)
            sync.dma_start(out.ap(), otile.ap()).then_inc(fsem, 16)
            sync.wait_ge(fsem, 16)

        @block.gpsimd
        def _(gpsimd):
            for j in range(T):
                if not tile_on_dve(j):
                    tree_reduce(gpsimd, j, rsem_p)

        @block.vector
        def _(vector):
            vector.wait_ge(psem, 48)
            for j in range(T):
                if j % OHB == 0:
                    b = j // OHB
                    nb = min(OHB, T - j)
                    if b >= 2:
                        vector.wait_ge(msem, OHB * (b - 1))
                    t = oh[b % 2]
                    vector.tensor_tensor(
                        bass.AP(t, 0, [[pstride(t), P], [P, nb], [1, P]]),
                        bass.AP(meta_sb, j, [[pstride(meta_sb), P], [1, nb], [0, P]]),
                        bass.AP(iota_sb, 0, [[pstride(iota_sb), P], [0, nb], [1, P]]),
                        mybir.AluOpType.is_equal,
                    ).then_inc(ohsem, 1)
                if tile_on_dve(j):
                    tree_reduce(vector, j, rsem_d)

            # ---- blend ----
            vector.wait_ge(msem, T)
            vc = [0]

            def chain(ins):
                ins.then_inc(vch, 1)
                vc[0] += 1
                vector.wait_ge(vch, vc[0])

            chain(vector.tensor_copy(cnt.ap(), col(psum_s, D)))
            chain(vector.tensor_scalar_max(rcp.ap(), cnt.ap(), 1.0))
            chain(vector.reciprocal(rcp.ap(), rcp.ap()))
            chain(vector.tensor_scalar(pres.ap(), cnt.ap(), 0.5, None,
                                       mybir.AluOpType.is_gt))
            if step_gt_warmup:
                chain(vector.tensor_reduce(
                    znorm.ap(), proto_sb.ap(),
                    axis=mybir.AxisListType.X, op=mybir.AluOpType.max,
                    apply_absolute_value=True,
                ))
                chain(vector.tensor_scalar(svec.ap(), znorm.ap(), 0.0, None,
                                           mybir.AluOpType.is_equal))
            else:
                chain(vector.memset(svec.ap(), 1.0))
            # svec = pres * (0.1 + 0.9*use_new)
            chain(vector.tensor_scalar(svec.ap(), svec.ap(), LAM, 1.0 - LAM,
                                       mybir.AluOpType.mult,
                                       mybir.AluOpType.add))
            chain(vector.tensor_tensor(svec.ap(), svec.ap(), pres.ap(),
                                       mybir.AluOpType.mult))
            chain(vector.tensor_scalar_mul(
                means.ap(), bass.AP(psum_s, 0, [[pstride(psum_s), P], [1, D]]),
                col(rcp, 0)))
            chain(vector.tensor_tensor(dtile.ap(), means.ap(), proto_sb.ap(),
                                       mybir.AluOpType.subtract))
            vector.scalar_tensor_tensor(
                otile.ap(), dtile.ap(), col(svec, 0), proto_sb.ap(),
                mybir.AluOpType.mult, mybir.AluOpType.add,
            ).then_inc(bsem, 1)

        @block.tensor
        def _(tensor):
            for j in range(T):
                wait_reduced(tensor, j)
                tensor.wait_ge(ohsem, j // OHB + 1)
                oh_t = oh[(j // OHB) % 2]
                tensor.matmul(
                    psum_s.ap(),
                    bass.AP(oh_t, (j % OHB) * P, [[pstride(oh_t), P], [1, P]]),
                    rhs[j % NR].ap(),
                    start=(j == 0), stop=(j == T - 1),
                ).then_inc(msem, 1)

    nc.compile()
    return nc


def shard_inputs(feature, label, prototype):
    """Returns (in_maps, cls_lists, T)."""
    import ml_dtypes
    bf16 = ml_dtypes.bfloat16

    counts = np.bincount(label, minlength=C)

    # greedy: biggest class -> least-loaded core (cap 128 classes/core)
    order_cls = np.argsort(-counts, kind="stable")
    core_load = np.zeros(N_CORES, dtype=np.int64)
    core_ncls = np.zeros(N_CORES, dtype=np.int64)
    cls_lists = [[] for _ in range(N_CORES)]
    nblk = (counts + GTOK - 1) // GTOK  # blocks per class
    for c in order_cls:
        k = min((k for k in range(N_CORES) if core_ncls[k] < P),
                key=lambda k: core_load[k])
        cls_lists[k].append(c)
        core_load[k] += nblk[c]
        core_ncls[k] += 1

    T = int(max(1, -(-core_load.max() // P)))
    cap_blk = T * P
    cap_tok = cap_blk * GTOK

    sort_order = np.argsort(label, kind="stable")
    starts = np.zeros(C + 1, dtype=np.int64)
    np.cumsum(counts, out=starts[1:])

    feat_bf = np.ascontiguousarray(feature, dtype=np.float32).astype(bf16)

    src_all = np.full(N_CORES * cap_tok, -1, dtype=np.int64)
    metas = []
    for k in range(N_CORES):
        base = k * cap_tok
        pos = 0
        mcls = np.zeros(cap_blk, dtype=np.float32)
        blk = 0
        for li, c in enumerate(cls_lists[k]):
            ncv = int(counts[c])
            if ncv:
                src_all[base + pos: base + pos + ncv] = \
                    sort_order[starts[c]: starts[c] + ncv]
            nb = int(nblk[c])
            if nb:
                mcls[blk: blk + nb] = li
            pos += nb * GTOK
            blk += nb
        metas.append(mcls)

    nblk_tot = N_CORES * cap_blk
    valid = src_all >= 0
    tok = np.zeros((nblk_tot * GTOK, D), dtype=bf16)
    tok[valid] = feat_bf[src_all[valid]]
    arr = np.empty((nblk_tot, GTOK, E), dtype=bf16)
    arr[:, :, :D] = tok.reshape(nblk_tot, GTOK, D)
    arr[:, :, D] = valid.reshape(nblk_tot, GTOK).astype(bf16)
    arr = arr.reshape(N_CORES, T * P, FW)

    proto32 = np.ascontiguousarray(prototype, dtype=np.float32)
    iota_arr = np.tile(np.arange(P, dtype=np.float32), (P, 1))
    in_maps = []
    for k in range(N_CORES):
        cl = np.asarray(cls_lists[k], dtype=np.int64)
        pk = np.zeros((P, D), dtype=np.float32)
        pk[: len(cl)] = proto32[cl]
        # block b=(tile j, partition p) -> meta[p, j]
        meta_k = np.ascontiguousarray(
            metas[k].reshape(T, P).T)
        in_maps.append({
            "feature": np.ascontiguousarray(arr[k]),
            "blk_meta": meta_k,
            "prototype": pk,
            "iota": iota_arr,
        })
    return in_maps, cls_lists, T


_NC_CACHE = {}


def run(inputs: dict, trace: bool = False):
    feature = np.asarray(inputs["feature"])
    label = np.asarray(inputs["label"], dtype=np.int64)
    prototype = np.asarray(inputs["prototype"])
    step = int(np.asarray(inputs["step"]))

    in_maps, cls_lists, T = shard_inputs(feature, label, prototype)
    key = (T, step > WARMUP_STEP)
    if key not in _NC_CACHE:
        _NC_CACHE[key] = build_nc(T, step > WARMUP_STEP)
    nc = _NC_CACHE[key]
    res = bass_utils.run_bass_kernel_spmd(
        nc, in_maps, core_ids=list(range(N_CORES)), trace=trace,
    )
    out = np.ascontiguousarray(prototype, dtype=np.float32).copy()
    for k in range(N_CORES):
        cl = np.asarray(cls_lists[k], dtype=np.int64)
        ok = np.asarray(res.results[k]["out"], dtype=np.float32)
        out[cl] = ok[: len(cl)]
    return out, res


def kernel(**inputs) -> np.ndarray:
    out, _ = run(inputs, trace=False)
    return out


# revision 13
# speedup vs baseline: 13.9296x; 1.2248x over previous
"""Trainium2 Bass kernel for the prototype-bank scatter-mean EMA update
(nn_Bank): class-sharded sorted-segment reduction across 8 NeuronCores.

Host (index/layout work only; all FP reduction arithmetic is on device):
  * argsort labels; assign each class to one core (greedy token balance,
    <=128 classes/core); concatenate each core's class segments, zero-
    padding every class to a multiple of GTOK tokens so each GTOK-token
    "block" is single-class.
  * feature blocks are packed feature-major [65, GTOK] in bf16: rows
    0..63 = the block's GTOK token features (transposed), row 64 = a
    1.0/0.0 valid-token indicator (so the same reduction that produces
    block feature sums also produces block counts).
  * per-core metadata: block -> local-class id; prototype rows for the
    core's classes; an iota row table.

Device, per core (T tiles; tile = 128 blocks = 128*GTOK tokens):
  1. Stream feature tiles [128, 65*GTOK] bf16 HBM->SBUF (sync queue).
  2. Block sums: tiles alternate between DVE tensor_reduce (true sums)
     and Pool avg-pool (sums/GTOK) -> rhs [128, 65] bf16.
  3. DVE builds one-hot lhsT [128 blocks, 128 local classes] bf16 via
     (iota == cls) * scale, scale = GTOK for Pool tiles (undoes the avg)
     and 1 for DVE tiles.
  4. PE: psum[cls, 65] += oh^T @ rhs, PSUM-accumulated over all T tiles
     -> per-class feature sums (cols 0..63) and counts (col 64).
  5. Blend: means = sums/max(cnt,1); out = proto + s*(means-proto) with
     s = present * (0.1 + 0.9*use_new)  [step>warmup branch].
  6. DMA out [128, D]; host scatters per-core rows back to [1000, 64].

No collective: every class is fully owned by one core.
"""

import numpy as np

import concourse.bacc as bacc
import concourse.bass as bass
import concourse.mybir as mybir
from concourse import bass_utils

C = 1000
D = 64
E = D + 1            # feature dims + count indicator (metadata only)
P = 128
GTOK = 32            # tokens per block (class padding granularity)
LAM = 0.9
WARMUP_STEP = 1000
N_CORES = 8
NB = 8               # feature tile buffers
NR = 4               # rhs / one-hot buffers
FW = D * GTOK        # free elems per feature tile partition
RW = 8 * D           # rhs cols handed to PE (8 partial sums x 64 dims)


OHB = 8              # one-hot batch size (tiles per build instruction)


def tile_on_dve(j: int) -> bool:
    # ~4:1 DVE:Pool split: contiguous bf16 tree-add ~1.5us/tile on DVE
    # (plus the one-hot batch builds) vs ~5-6.5us/tile on Pool
    return j % 5 != 4


def build_nc(T: int, step_gt_warmup: bool):
    f32 = mybir.dt.float32
    bf16 = mybir.dt.bfloat16

    dcount = [0] * (T + 1)  # dcount[j+1] = #DVE tiles among 0..j
    pcount = [0] * (T + 1)
    for j in range(T):
        dcount[j + 1] = dcount[j] + (1 if tile_on_dve(j) else 0)
        pcount[j + 1] = pcount[j] + (0 if tile_on_dve(j) else 1)

    nc = bacc.Bacc("TRN2", target_bir_lowering=False, debug=False,
                   num_devices=N_CORES)

    feat = nc.dram_tensor("feature", [T * P, FW], bf16, kind="ExternalInput")
    meta = nc.dram_tensor("blk_meta", [P, T], f32, kind="ExternalInput")
    meta2 = nc.dram_tensor("blk_cnt", [P, T], bf16, kind="ExternalInput")
    proto = nc.dram_tensor("prototype", [P, D], f32, kind="ExternalInput")
    iota = nc.dram_tensor("iota", [P, P], f32, kind="ExternalInput")
    out = nc.dram_tensor("out", [P, D], f32, kind="ExternalOutput")

    ftiles = [nc.alloc_sbuf_tensor(f"ftile{b}", [P, FW], bf16) for b in range(NB)]
    iota_sb = nc.alloc_sbuf_tensor("iota_sb", [P, P], f32)
    meta_sb = nc.alloc_sbuf_tensor("meta_sb", [P, T], f32)
    meta2_sb = nc.alloc_sbuf_tensor("meta2_sb", [P, T], bf16)
    proto_sb = nc.alloc_sbuf_tensor("proto_sb", [P, D], f32)
    rhs = [nc.alloc_sbuf_tensor(f"rhs{i}", [P, RW], bf16) for i in range(NR)]
    oh = [nc.alloc_sbuf_tensor(f"oh{i}", [P, OHB * P], bf16) for i in range(2)]
    cnt = nc.alloc_sbuf_tensor("cnt", [P, 1], f32)
    rcp = nc.alloc_sbuf_tensor("rcp", [P, 1], f32)
    pres = nc.alloc_sbuf_tensor("pres", [P, 1], f32)
    znorm = nc.alloc_sbuf_tensor("znorm", [P, 1], f32)
    svec = nc.alloc_sbuf_tensor("svec", [P, 1], f32)
    means = nc.alloc_sbuf_tensor("means", [P, D], f32)
    dtile = nc.alloc_sbuf_tensor("dtile", [P, D], f32)
    otile = nc.alloc_sbuf_tensor("otile", [P, D], f32)

    psum_s = nc.alloc_psum_tensor("psum_s", [P, RW], f32)
    psum_c = nc.alloc_psum_tensor("psum_c", [P, 1], f32)
    sums = nc.alloc_sbuf_tensor("sums", [P, D], f32)

    lsems = [nc.alloc_semaphore(f"lsem{b}") for b in range(NB)]
    rsem_d = nc.alloc_semaphore("rsem_d")  # DVE reduces done
    rsem_p = nc.alloc_semaphore("rsem_p")  # Pool reduces done
    ohsem = nc.alloc_semaphore("ohsem")    # one-hots built
    msem = nc.alloc_semaphore("msem")      # matmuls done (1 per tile)
    psem = nc.alloc_semaphore("psem")      # preamble loads (3 x 16)
    bsem = nc.alloc_semaphore("bsem")      # blend done
    fsem = nc.alloc_semaphore("fsem")      # out store done
    vch = nc.alloc_semaphore("vch")        # blend chain

    def pstride(t):
        return t.ap().ap[0][0]

    def feat_tile_ap(j):
        return bass.AP(feat, j * P * FW, [[FW, P], [1, FW]])

    def ftile_red_ap(b):
        t = ftiles[b]
        return bass.AP(t, 0, [[pstride(t), P], [GTOK, E], [1, GTOK]])

    def col(t, c, w=1):
        return bass.AP(t, c, [[pstride(t), P], [1, w]])

    def wait_reduced(eng, j):
        """Wait until tile j's reduce has retired."""
        if tile_on_dve(j):
            eng.wait_ge(rsem_d, dcount[j + 1])
        else:
            eng.wait_ge(rsem_p, pcount[j + 1])

    def tree_reduce(eng, j, rsem):
        """In-place bf16 pairwise tree: ftile[b] [P, E, GTOK] summed over
        GTOK -> rhs[j%NR] [P, E]. Consumes (overwrites) the ftile."""
        b = j % NB
        t = ftiles[b]
        eng.wait_ge(lsems[b], 16 * (j // NB + 1))
        h = FW // 2
        eng.tensor_tensor(
            bass.AP(t, 0, [[pstride(t), P], [1, h]]),
            bass.AP(t, 0, [[pstride(t), P], [1, h]]),
            bass.AP(t, h, [[pstride(t), P], [1, h]]),
            mybir.AluOpType.add,
        )
        if j >= NR:
            eng.wait_ge(msem, j - NR + 1)  # rhs slot freed by matmul j-NR
        eng.tensor_tensor(
            rhs[j % NR].ap(),
            bass.AP(t, 0, [[pstride(t), P], [1, RW]]),
            bass.AP(t, RW, [[pstride(t), P], [1, RW]]),
            mybir.AluOpType.add,
        ).then_inc(rsem, 1)

    with nc.allow_low_precision("bf16 block sums; exact count col"), \
            nc.Block() as block:

        @block.scalar
        def _(scalar):
            scalar.dma_start(iota_sb.ap(), iota.ap()).then_inc(psem, 16)
            scalar.dma_start(meta_sb.ap(), meta.ap()).then_inc(psem, 16)
            scalar.dma_start(proto_sb.ap(), proto.ap()).then_inc(psem, 16)
            scalar.dma_start(meta2_sb.ap(), meta2.ap()).then_inc(psem, 16)

        @block.sync
        def _(sync):
            for j in range(T):
                b = j % NB
                if j >= NB:
                    wait_reduced(sync, j - NB)
                sync.dma_start(ftiles[b].ap(), feat_tile_ap(j)).then_inc(lsems[b], 16)
            sync.wait_ge(bsem, 1)
            sync.dma_start(out.ap(), otile.ap()).then_inc(fsem, 16)
            sync.wait_ge(fsem, 16)

        @block.gpsimd
        def _(gpsimd):
            for j in range(T):
                if not tile_on_dve(j):
                    tree_reduce(gpsimd, j, rsem_p)

        @block.vector
        def _(vector):
            vector.wait_ge(psem, 64)
            for j in range(T):
                if j % OHB == 0:
                    b = j // OHB
                    nb = min(OHB, T - j)
                    if b >= 2:
                        vector.wait_ge(msem, OHB * (b - 1))
                    t = oh[b % 2]
                    vector.tensor_tensor(
                        bass.AP(t, 0, [[pstride(t), P], [P, nb], [1, P]]),
                        bass.AP(meta_sb, j, [[pstride(meta_sb), P], [1, nb], [0, P]]),
                        bass.AP(iota_sb, 0, [[pstride(iota_sb), P], [0, nb], [1, P]]),
                        mybir.AluOpType.is_equal,
                    ).then_inc(ohsem, 1)
                if tile_on_dve(j):
                    tree_reduce(vector, j, rsem_d)

            # ---- blend ----
            vector.wait_ge(msem, T)
            vc = [0]

            def chain(ins):
                ins.then_inc(vch, 1)
                vc[0] += 1
                vector.wait_ge(vch, vc[0])

            chain(vector.tensor_reduce(
                sums.ap(),
                bass.AP(psum_s, 0, [[pstride(psum_s), P], [1, D], [D, 8]]),
                axis=mybir.AxisListType.X, op=mybir.AluOpType.add,
            ))
            chain(vector.tensor_copy(cnt.ap(), psum_c.ap()))
            chain(vector.tensor_scalar_max(rcp.ap(), cnt.ap(), 1.0))
            chain(vector.reciprocal(rcp.ap(), rcp.ap()))
            chain(vector.tensor_scalar(pres.ap(), cnt.ap(), 0.5, None,
                                       mybir.AluOpType.is_gt))
            if step_gt_warmup:
                chain(vector.tensor_reduce(
                    znorm.ap(), proto_sb.ap(),
                    axis=mybir.AxisListType.X, op=mybir.AluOpType.max,
                    apply_absolute_value=True,
                ))
                chain(vector.tensor_scalar(svec.ap(), znorm.ap(), 0.0, None,
                                           mybir.AluOpType.is_equal))
            else:
                chain(vector.memset(svec.ap(), 1.0))
            # svec = pres * (0.1 + 0.9*use_new)
            chain(vector.tensor_scalar(svec.ap(), svec.ap(), LAM, 1.0 - LAM,
                                       mybir.AluOpType.mult,
                                       mybir.AluOpType.add))
            chain(vector.tensor_tensor(svec.ap(), svec.ap(), pres.ap(),
                                       mybir.AluOpType.mult))
            chain(vector.tensor_scalar_mul(means.ap(), sums.ap(), col(rcp, 0)))
            chain(vector.tensor_tensor(dtile.ap(), means.ap(), proto_sb.ap(),
                                       mybir.AluOpType.subtract))
            vector.scalar_tensor_tensor(
                otile.ap(), dtile.ap(), col(svec, 0), proto_sb.ap(),
                mybir.AluOpType.mult, mybir.AluOpType.add,
            ).then_inc(bsem, 1)

        @block.tensor
        def _(tensor):
            tensor.wait_ge(psem, 64)
            for j in range(T):
                wait_reduced(tensor, j)
                tensor.wait_ge(ohsem, j // OHB + 1)
                oh_t = oh[(j // OHB) % 2]
                oh_ap = bass.AP(oh_t, (j % OHB) * P, [[pstride(oh_t), P], [1, P]])
                tensor.matmul(
                    psum_s.ap(), oh_ap, rhs[j % NR].ap(),
                    start=(j == 0), stop=(j == T - 1),
                )
                tensor.matmul(
                    psum_c.ap(), oh_ap,
                    bass.AP(meta2_sb, j, [[pstride(meta2_sb), P], [1, 1]]),
                    start=(j == 0), stop=(j == T - 1),
                ).then_inc(msem, 1)

    nc.compile()
    return nc


def shard_inputs(feature, label, prototype):
    """Returns (in_maps, cls_lists, T)."""
    import ml_dtypes
    bf16 = ml_dtypes.bfloat16

    counts = np.bincount(label, minlength=C)

    # greedy: biggest class -> least-loaded core (cap 128 classes/core)
    order_cls = np.argsort(-counts, kind="stable")
    core_load = np.zeros(N_CORES, dtype=np.int64)
    core_ncls = np.zeros(N_CORES, dtype=np.int64)
    cls_lists = [[] for _ in range(N_CORES)]
    nblk = (counts + GTOK - 1) // GTOK  # blocks per class
    for c in order_cls:
        k = min((k for k in range(N_CORES) if core_ncls[k] < P),
                key=lambda k: core_load[k])
        cls_lists[k].append(c)
        core_load[k] += nblk[c]
        core_ncls[k] += 1

    T = int(max(1, -(-core_load.max() // P)))
    cap_blk = T * P
    cap_tok = cap_blk * GTOK

    sort_order = np.argsort(label, kind="stable")
    starts = np.zeros(C + 1, dtype=np.int64)
    np.cumsum(counts, out=starts[1:])

    feat_bf = np.ascontiguousarray(feature, dtype=np.float32).astype(bf16)

    src_all = np.full(N_CORES * cap_tok, -1, dtype=np.int64)
    metas = []
    for k in range(N_CORES):
        base = k * cap_tok
        pos = 0
        mcls = np.zeros(cap_blk, dtype=np.float32)
        mcnt = np.zeros(cap_blk, dtype=np.float32)
        blk = 0
        for li, c in enumerate(cls_lists[k]):
            ncv = int(counts[c])
            if ncv:
                src_all[base + pos: base + pos + ncv] = \
                    sort_order[starts[c]: starts[c] + ncv]
            nb = int(nblk[c])
            if nb:
                mcls[blk: blk + nb] = li
                mcnt[blk: blk + nb] = GTOK
                mcnt[blk + nb - 1] = ncv - (nb - 1) * GTOK
            pos += nb * GTOK
            blk += nb
        metas.append((mcls, mcnt))

    nblk_tot = N_CORES * cap_blk
    valid = src_all >= 0
    tok = np.zeros((nblk_tot * GTOK, D), dtype=bf16)
    tok[valid] = feat_bf[src_all[valid]]
    arr = tok.reshape(N_CORES, T * P, FW)

    proto32 = np.ascontiguousarray(prototype, dtype=np.float32)
    iota_arr = np.tile(np.arange(P, dtype=np.float32), (P, 1))
    in_maps = []
    for k in range(N_CORES):
        cl = np.asarray(cls_lists[k], dtype=np.int64)
        pk = np.zeros((P, D), dtype=np.float32)
        pk[: len(cl)] = proto32[cl]
        # block b=(tile j, partition p) -> meta[p, j]
        meta_k = np.ascontiguousarray(metas[k][0].reshape(T, P).T)
        meta2_k = np.ascontiguousarray(
            metas[k][1].reshape(T, P).T.astype(bf16))
        in_maps.append({
            "feature": np.ascontiguousarray(arr[k]),
            "blk_meta": meta_k,
            "blk_cnt": meta2_k,
            "prototype": pk,
            "iota": iota_arr,
        })
    return in_maps, cls_lists, T


_NC_CACHE = {}


def run(inputs: dict, trace: bool = False):
    feature = np.asarray(inputs["feature"])
    label = np.asarray(inputs["label"], dtype=np.int64)
    prototype = np.asarray(inputs["prototype"])
    step = int(np.asarray(inputs["step"]))

    in_maps, cls_lists, T = shard_inputs(feature, label, prototype)
    key = (T, step > WARMUP_STEP)
    if key not in _NC_CACHE:
        _NC_CACHE[key] = build_nc(T, step > WARMUP_STEP)
    nc = _NC_CACHE[key]
    res = bass_utils.run_bass_kernel_spmd(
        nc, in_maps, core_ids=list(range(N_CORES)), trace=trace,
    )
    out = np.ascontiguousarray(prototype, dtype=np.float32).copy()
    for k in range(N_CORES):
        cl = np.asarray(cls_lists[k], dtype=np.int64)
        ok = np.asarray(res.results[k]["out"], dtype=np.float32)
        out[cl] = ok[: len(cl)]
    return out, res


def kernel(**inputs) -> np.ndarray:
    out, _ = run(inputs, trace=False)
    return out
